# revision 1
# baseline (speedup 1.0000x reference)
"""VMamba-style block (LN -> in_proj -> dwconv3x3 -> selective scan -> gated
out_proj -> MLP) for Trainium2, data-parallel over 8 NeuronCores.

8 independent sequences (x: 4 batches, y: 4 batches), one per core, zero
collectives.  Per core: feature-major layouts ([feature, token]) so every
matmul contracts over the partition dim; the Mamba scan runs as 16 per-state
hardware linear scans (tensor_tensor_scan) over [d, t] lanes, split across
the Vector and GpSimd engines, with exp(A_n * dt) from the Scalar engine.
"""

import numpy as np
import ml_dtypes

import concourse.bass as bass
import concourse.mybir as mybir
import concourse.tile as tile
from concourse.bass_utils import run_bass_kernel_spmd
from concourse.masks import make_identity

# ---------------------------------------------------------------- constants
L = 1024          # H*W sequence length
DIM = 384
DI = 768          # d_inner
N = 16            # d_state
DTR = 24          # dt_rank
HID = 192         # mlp hidden
EPS = 1e-5
F32 = mybir.dt.float32
BF16 = mybir.dt.bfloat16
BF = ml_dtypes.bfloat16
AX = mybir.AxisListType
OP = mybir.AluOpType
AF = mybir.ActivationFunctionType

_CACHE = {}


def _install_tilefix():
    """This walrus build rejects CTRL instructions (Drain/NoOp on SP) with
    more than ~2 sync waits.  Split the TileContext tail-drain waits across
    one SP NoOp per semaphore."""
    if getattr(tile.TileContext, "_drainfix", False):
        return
    from bass_rust import ScopedClock

    def _drain_and_barrier(self, tick_clock, wait_clock):
        nop_inst = self.nc.sync.nop(nofuse=True)
        wait_clock.add_sem_waits(
            nop_inst.ins, ScopedClock({None: tick_clock.global_clock})
        )
        si = nop_inst.ins.sync_info
        waits = list(si.on_wait) if si is not None else []
        if len(waits) > 1:
            nop_inst.ins.sync_info = mybir.SyncInfo(
                on_wait=[waits[0]], on_update=list(si.on_update)
            )
            for w in waits[1:]:
                extra = self.nc.sync.nop(nofuse=True)
                extra.ins.sync_info = mybir.SyncInfo(on_wait=[w], on_update=[])
        self.nc.sync.drain()
        self.nc.all_engine_barrier()
        assert self.sems is not None
        popped = self.nc._tile_sem_poison_stack.pop()
        assert popped is self._sem_poison
        self.nc.clear_and_free_semaphores(list(self.sems.allocated().values()))
        self.nc.all_engine_barrier()

    tile.TileContext._drain_and_barrier = _drain_and_barrier
    tile.TileContext._drainfix = True


_WSPLIT = [0]


def _split_excess_waits(nc):
    """This walrus build allows at most ~2 sync waits per compute
    instruction (1 for CTRL ops).  Move excess waits onto same-engine
    NoOps inserted immediately before the instruction."""
    for f in nc.m.functions:
        for bb in f.blocks:
            il = bb.instructions
            i = 0
            while i < len(il):
                inst = il[i]
                si = getattr(inst, "sync_info", None)
                if si is not None and si.on_wait:
                    cap = 1
                    waits = list(si.on_wait)
                    if len(waits) > cap:
                        keep = waits[len(waits) - cap:]
                        extra = waits[:len(waits) - cap]
                        inst.sync_info = mybir.SyncInfo(
                            on_wait=keep, on_update=list(si.on_update)
                        )
                        for w in extra:
                            nop = mybir.InstNoOp(
                                name=f"wsplit_{_WSPLIT[0]}", engine=inst.engine
                            )
                            _WSPLIT[0] += 1
                            nop.sync_info = mybir.SyncInfo(on_wait=[w], on_update=[])
                            il.insert(i, nop)
                            i += 1
                i += 1


def _build_program(a_coefs):
    """Build the single-core SPMD program. a_coefs: 16 python floats, the
    (d-independent) continuous-time decay rates A[n]."""
    nc = bass.Bass()
    P = lambda name, shape, dt=F32: nc.declare_dram_parameter(
        name, list(shape), dt, isOutput=False
    )
    u_d = P("u", (L, DIM))                       # this core's sequence
    w_in_t = P("w_in_t", (DIM, 2 * DI), BF16)    # in_proj_w.T
    b_in = P("b_in", (2 * DI, 1))
    convw = P("convw", (DI, 9))                  # 9 taps per channel
    convb = P("convb", (DI, 1))
    xp_t = P("xp_t", (DI, DTR + 2 * N), BF16)    # x_proj_w.T
    dtp_t = P("dtp_t", (DTR, DI), BF16)          # dt_proj_w.T
    dtp_b = P("dtp_b", (DI, 1))
    d_skip = P("d_skip", (DI, 1))
    onw = P("onw", (DI, 1))
    onb = P("onb", (DI, 1))
    op_t = P("op_t", (DI, DIM), BF16)            # out_proj_w.T
    ln1w = P("ln1w", (DIM, 1))
    ln1b = P("ln1b", (DIM, 1))
    ln2w = P("ln2w", (DIM, 1))
    ln2b = P("ln2b", (DIM, 1))
    fc1_t = P("fc1_t", (DIM, HID), BF16)
    fc1_b = P("fc1_b", (HID, 1))
    fc2_t = P("fc2_t", (HID, DIM), BF16)
    fc2_b = P("fc2_b", (DIM, 1))
    o_d = nc.declare_dram_parameter("o", [L, DIM], F32, isOutput=True)

    TT = L // 128   # 8 token tiles
    KD = DIM // 128  # 3
    KI = DI // 128   # 6
    CH = L // 512    # 2 moving chunks

    from contextlib import ExitStack

    with tile.TileContext(nc) as tc, ExitStack() as es:
        persist = es.enter_context(tc.tile_pool(name="persist", bufs=1))
        pp_mm = es.enter_context(tc.tile_pool(name="pp_mm", bufs=4, space="PSUM"))
        pp_t = es.enter_context(tc.tile_pool(name="pp_t", bufs=2, space="PSUM"))
        dram = es.enter_context(tc.tile_pool(name="dram", bufs=1, space="DRAM"))

        ident_f = persist.tile([128, 128], F32, tag="ident_f", name="ident_f")
        make_identity(nc, ident_f)
        ident_b = persist.tile([128, 128], BF16, tag="ident_b", name="ident_b")
        make_identity(nc, ident_b)
        eps_t = persist.tile([128, 1], F32, tag="eps", name="eps")
        nc.vector.memset(eps_t, EPS)

        # ---- load weights
        def wload(dram_ap, shape, dt, tag, pieces=1):
            ts_ = []
            for k in range(pieces):
                t = persist.tile([shape[0] // pieces, shape[1]], dt, tag=f"{tag}{k}", name=f"{tag}{k}")
                nc.sync.dma_start(
                    out=t, in_=dram_ap[k * shape[0] // pieces:(k + 1) * shape[0] // pieces, :]
                )
                ts_.append(t)
            return ts_

        w_in_s = wload(w_in_t, (DIM, 2 * DI), BF16, "w_in", KD)      # 3x[128,1536]
        b_in_s = wload(b_in, (2 * DI, 1), F32, "b_in", 2 * KI)       # 12x[128,1]
        convw_s = wload(convw, (DI, 9), F32, "convw", KI)
        convb_s = wload(convb, (DI, 1), F32, "convb", KI)
        xp_s = wload(xp_t, (DI, DTR + 2 * N), BF16, "xp", KI)
        dtp_s = wload(dtp_t, (DTR, DI), BF16, "dtp", 1)              # [24,768]
        dtpb_s = wload(dtp_b, (DI, 1), F32, "dtpb", KI)
        dsk_s = wload(d_skip, (DI, 1), F32, "dsk", KI)
        onw_s = wload(onw, (DI, 1), F32, "onw", KI)
        onb_s = wload(onb, (DI, 1), F32, "onb", KI)
        op_s = wload(op_t, (DI, DIM), BF16, "op", KI)
        ln1w_s = wload(ln1w, (DIM, 1), F32, "ln1w", KD)
        ln1b_s = wload(ln1b, (DIM, 1), F32, "ln1b", KD)
        ln2w_s = wload(ln2w, (DIM, 1), F32, "ln2w", KD)
        ln2b_s = wload(ln2b, (DIM, 1), F32, "ln2b", KD)
        fc1_s = wload(fc1_t, (DIM, HID), BF16, "fc1", KD)
        fc1b_s = wload(fc1_b, (HID, 1), F32, "fc1b", 2)              # 2x[96,1]
        fc2_s = wload(fc2_t, (HID, DIM), BF16, "fc2", 2)             # 2x[96,384]
        fc2b_s = wload(fc2_b, (DIM, 1), F32, "fc2b", KD)

        # persistent activations
        u_tm = [persist.tile([128, DIM], F32, tag=f"u_tm{i}", name=f"u_tm{i}") for i in range(TT)]
        ulnT = [persist.tile([128, L], BF16, tag=f"ulnT{j}", name=f"ulnT{j}") for j in range(KD)]
        zsil = [persist.tile([128, L], BF16, tag=f"zsil{s}", name=f"zsil{s}") for s in range(KI)]
        xs_b = [persist.tile([128, L], BF16, tag=f"xsb{s}", name=f"xsb{s}") for s in range(KI)]
        dt_b = [persist.tile([128, L], BF16, tag=f"dtb{s}", name=f"dtb{s}") for s in range(KI)]
        dtx = [persist.tile([128, L], BF16, tag=f"dtx{s}", name=f"dtx{s}") for s in range(KI)]
        y_ac = [persist.tile([128, L], F32, tag=f"yac{s}", name=f"yac{s}") for s in range(KI)]

        for i in range(TT):
            nc.sync.dma_start(out=u_tm[i], in_=u_d[i * 128:(i + 1) * 128, :])

        # ---------------- LN1 (token-major) + transpose to feature-major
        with tc.tile_pool(name="ln1", bufs=5) as pool:
            for i in range(TT):
                st = pool.tile([128, 6], F32, tag="st", name="st")
                nc.vector.bn_stats(out=st, in_=u_tm[i])
                mv = pool.tile([128, 2], F32, tag="mv", name="mv")
                nc.vector.bn_aggr(out=mv, in_=st)
                nc.scalar.activation(
                    out=mv[:, 1:2], in_=mv[:, 1:2], func=AF.Sqrt, bias=eps_t
                )
                nc.vector.reciprocal(out=mv[:, 1:2], in_=mv[:, 1:2])
                uln = pool.tile([128, DIM], BF16, tag="uln", name="uln")
                nc.vector.tensor_scalar(
                    out=uln, in0=u_tm[i], scalar1=mv[:, 0:1], scalar2=mv[:, 1:2],
                    op0=OP.subtract, op1=OP.mult,
                )
                for j in range(KD):
                    pt = pp_t.tile([128, 128], BF16, tag="tpb", name="tpb")
                    nc.tensor.transpose(pt, uln[:, j * 128:(j + 1) * 128], ident_b)
                    nc.scalar.activation(
                        out=ulnT[j][:, i * 128:(i + 1) * 128], in_=pt, func=AF.Copy
                    )
            # ln1 w/b in feature-major (per-partition scalars)
            for j in range(KD):
                nc.vector.tensor_scalar(
                    out=ulnT[j], in0=ulnT[j], scalar1=ln1w_s[j], scalar2=ln1b_s[j],
                    op0=OP.mult, op1=OP.add,
                )

        # ---------------- in_proj -> xp (padded for conv) and z_silu
        # padded conv input: 34x34 per partition
        xpool_cm = tc.tile_pool(name="xpadp", bufs=1)
        xpool = xpool_cm.__enter__()
        xpad = [xpool.tile([128, 34 * 34], BF16, tag=f"xpad{s}", name=f"xpad{s}") for s in range(KI)]
        for s in range(KI):
            nc.vector.memset(xpad[s], 0.0)
        with tc.tile_pool(name="inproj", bufs=6) as pool:
            for m in range(2 * KI):          # 12 feature tiles of 128
                for c in range(CH):
                    ps = pp_mm.tile([128, 512], F32, tag="mm", name="mm")
                    for k in range(KD):
                        nc.tensor.matmul(
                            ps, w_in_s[k][:, m * 128:(m + 1) * 128],
                            ulnT[k][:, c * 512:(c + 1) * 512],
                            start=(k == 0), stop=(k == KD - 1),
                        )
                    if m < KI:
                        # xp rows -> padded sbuf (+bias), h-rows 16c..16c+16
                        dst = xpad[m].rearrange("p (h w) -> p h w", h=34)
                        nc.vector.tensor_scalar(
                            out=dst[:, 1 + 16 * c:1 + 16 * (c + 1), 1:33],
                            in0=ps.rearrange("p (h w) -> p h w", h=16),
                            scalar1=b_in_s[m], scalar2=None, op0=OP.add,
                        )
                    else:
                        nc.scalar.activation(
                            out=zsil[m - KI][:, c * 512:(c + 1) * 512], in_=ps,
                            func=AF.Silu, bias=b_in_s[m],
                        )

        # ---------------- depthwise 3x3 conv + silu -> xs
        with tc.tile_pool(name="conv", bufs=4) as pool:
            for s in range(KI):
                xv = xpad[s].rearrange("p (h w) -> p h w", h=34)
                acc = pool.tile([128, L], BF16, tag="acc", name="acc")
                # center tap (k=4) * w + bias
                nc.vector.tensor_scalar(
                    out=acc, in0=xv[:, 1:33, 1:33], scalar1=convw_s[s][:, 4:5],
                    scalar2=convb_s[s], op0=OP.mult, op1=OP.add,
                )
                k = 0
                for dh in (-1, 0, 1):
                    for dw in (-1, 0, 1):
                        if k != 4:
                            eng = nc.vector
                            acc2 = pool.tile([128, L], BF16, tag="acc", name="acc")
                            eng.scalar_tensor_tensor(
                                out=acc2,
                                in0=xv[:, 1 + dh:33 + dh, 1 + dw:33 + dw],
                                scalar=convw_s[s][:, k:k + 1], in1=acc,
                                op0=OP.mult, op1=OP.add,
                            )
                            acc = acc2
                        k += 1
                nc.scalar.activation(out=xs_b[s], in_=acc, func=AF.Silu)
        xpool_cm.__exit__(None, None, None)

        # ---------------- x_proj -> x_dbl [56, L]; dt_proj -> dt
        xdbl = persist.tile([DTR + 2 * N, L], F32, tag="xdbl", name="xdbl")
        xdbl_b = persist.tile([DTR + 2 * N, L], BF16, tag="xdbl_b", name="xdbl_b")
        with tc.tile_pool(name="xproj", bufs=6) as pool:
            for c in range(CH):
                ps = pp_mm.tile([128, 512], F32, tag="mm", name="mm")
                for k in range(KI):
                    nc.tensor.matmul(
                        ps[:DTR + 2 * N], xp_s[k], xs_b[k][:, c * 512:(c + 1) * 512],
                        start=(k == 0), stop=(k == KI - 1),
                    )
                nc.scalar.activation(
                    out=xdbl[:, c * 512:(c + 1) * 512], in_=ps[:DTR + 2 * N],
                    func=AF.Copy,
                )
            nc.vector.tensor_copy(out=xdbl_b, in_=xdbl)
            for m in range(KI):
                for c in range(CH):
                    ps = pp_mm.tile([128, 512], F32, tag="mm", name="mm")
                    nc.tensor.matmul(
                        ps, dtp_s[0][:, m * 128:(m + 1) * 128],
                        xdbl_b[:DTR, c * 512:(c + 1) * 512],
                        start=True, stop=True,
                    )
                    et = pool.tile([128, 512], F32, tag="et", name="et")
                    nc.scalar.activation(
                        out=et, in_=ps, func=AF.Exp, bias=dtpb_s[m],
                    )
                    # softplus(v) = ln(1 + exp(v)); v in [-5.2, -4.0] here
                    nc.scalar.activation(
                        out=dt_b[m][:, c * 512:(c + 1) * 512], in_=et,
                        func=AF.Ln, bias=1.0,
                    )
            for s in range(KI):
                nc.vector.tensor_mul(out=dtx[s], in0=dt_b[s], in1=xs_b[s])

        # ---------------- B/C broadcast rows via DRAM bounce
        bc_d = dram.tile([2 * N, L], BF16, tag="bc", name="bc")
        nc.sync.dma_start(out=bc_d, in_=xdbl_b[DTR:, :])

        # ---------------- the 16-state scan
        for s in range(KI):
            nc.vector.memset(y_ac[s], 0.0)
        with tc.tile_pool(name="scan", bufs=4) as pool:
            for n in range(N):
                bb = pool.tile([128, L], BF16, tag="bb", name="bb")
                nc.sync.dma_start(
                    out=bb, in_=bc_d[n:n + 1, :].to_broadcast((128, L))
                )
                cb = pool.tile([128, L], BF16, tag="cb", name="cb")
                nc.sync.dma_start(
                    out=cb, in_=bc_d[N + n:N + n + 1, :].to_broadcast((128, L))
                )
                for s in range(KI):
                    da = pool.tile([128, L], F32, tag="da", name="da", bufs=5)
                    nc.scalar.activation(
                        out=da, in_=dt_b[s], func=AF.Exp, scale=float(a_coefs[n])
                    )
                    dbu = pool.tile([128, L], BF16, tag="dbu", name="dbu")
                    nc.vector.tensor_mul(out=dbu, in0=dtx[s], in1=bb)
                    h = pool.tile([128, L], BF16, tag="h", name="h", bufs=5)
                    seng = nc.vector
                    seng.tensor_tensor_scan(
                        out=h, data0=da, data1=dbu, initial=0.0,
                        op0=OP.mult, op1=OP.add,
                    )
                    hc = pool.tile([128, L], BF16, tag="hc", name="hc")
                    heng = nc.vector if s < 5 else nc.gpsimd
                    heng.tensor_mul(out=hc, in0=h, in1=cb)
                    nc.gpsimd.dma_start(out=y_ac[s], in_=hc, accum_op=OP.add)

        # ---------------- y += xs*D ; out_norm (feature-major LN over 768)
        ones_f = persist.tile([128, 1], F32, tag="ones_f", name="ones_f")
        nc.vector.memset(ones_f, 1.0)
        ones_row = persist.tile([1, 128], F32, tag="ones_row", name="ones_row")
        nc.vector.memset(ones_row, 1.0)
        ssum = persist.tile([1, L], F32, tag="ssum", name="ssum")
        ssq = persist.tile([1, L], F32, tag="ssq", name="ssq")
        mean_b = persist.tile([128, L], F32, tag="mean_b", name="mean_b")
        rstd_b = persist.tile([128, L], F32, tag="rstd_b", name="rstd_b")
        with tc.tile_pool(name="onorm", bufs=6) as pool:
            for s in range(KI):
                nc.vector.scalar_tensor_tensor(
                    out=y_ac[s], in0=xs_b[s], scalar=dsk_s[s], in1=y_ac[s],
                    op0=OP.mult, op1=OP.add,
                )
            for c in range(CH):
                ps = pp_mm.tile([128, 512], F32, tag="mm", name="mm")
                for k in range(KI):
                    nc.tensor.matmul(
                        ps[:1], ones_f[:, :], y_ac[k][:, c * 512:(c + 1) * 512],
                        start=(k == 0), stop=(k == KI - 1),
                    )
                nc.vector.tensor_copy(out=ssum[:, c * 512:(c + 1) * 512], in_=ps[:1])
            for c in range(CH):
                ps = pp_mm.tile([128, 512], F32, tag="mm", name="mm")
                for k in range(KI):
                    ysq = pool.tile([128, 512], F32, tag="ysq", name="ysq")
                    nc.vector.tensor_mul(
                        out=ysq, in0=y_ac[k][:, c * 512:(c + 1) * 512],
                        in1=y_ac[k][:, c * 512:(c + 1) * 512],
                    )
                    nc.tensor.matmul(
                        ps[:1], ones_f[:, :], ysq,
                        start=(k == 0), stop=(k == KI - 1),
                    )
                nc.vector.tensor_copy(out=ssq[:, c * 512:(c + 1) * 512], in_=ps[:1])
            # mean/rstd on [1, L]
            nc.scalar.mul(out=ssum, in_=ssum, mul=1.0 / DI)          # mean
            nc.scalar.mul(out=ssq, in_=ssq, mul=1.0 / DI)            # E[y^2]
            msq = pool.tile([1, L], F32, tag="msq", name="msq")
            nc.vector.tensor_mul(out=msq, in0=ssum, in1=ssum)
            nc.vector.tensor_sub(out=msq, in0=ssq, in1=msq)          # var
            nc.scalar.activation(out=msq, in_=msq, func=AF.Sqrt, bias=eps_t[:1])
            nc.vector.reciprocal(out=msq, in_=msq)                   # rstd
            # broadcast mean/rstd to 128 partitions via PE outer product
            for src, dst in ((ssum, mean_b), (msq, rstd_b)):
                for c in range(CH):
                    pb = pp_mm.tile([128, 512], F32, tag="mm", name="mm")
                    nc.tensor.matmul(
                        pb, ones_row, src[:, c * 512:(c + 1) * 512],
                        start=True, stop=True,
                    )
                    nc.scalar.activation(
                        out=dst[:, c * 512:(c + 1) * 512], in_=pb, func=AF.Copy
                    )
            gate = zsil
            for s in range(KI):
                t1 = pool.tile([128, L], F32, tag="t1", name="t1")
                neng = nc.vector if s < 3 else nc.gpsimd
                neng.tensor_sub(out=t1, in0=y_ac[s], in1=mean_b)
                neng.tensor_mul(out=t1, in0=t1, in1=rstd_b)
                nc.vector.tensor_scalar(
                    out=t1, in0=t1, scalar1=onw_s[s], scalar2=onb_s[s],
                    op0=OP.mult, op1=OP.add,
                )
                nc.vector.tensor_mul(out=gate[s], in0=t1, in1=zsil[s])  # in-place over zsil

        # ---------------- out_proj -> mix ; a = u + mixT (token-major, in-place)
        a_tm = u_tm
        with tc.tile_pool(name="oproj", bufs=6) as pool:
            for m in range(KD):
                mixs = pool.tile([128, L], F32, tag="mixs", name="mixs")
                for c in range(CH):
                    ps = pp_mm.tile([128, 512], F32, tag="mm", name="mm")
                    for k in range(KI):
                        nc.tensor.matmul(
                            ps, op_s[k][:, m * 128:(m + 1) * 128],
                            gate[k][:, c * 512:(c + 1) * 512],
                            start=(k == 0), stop=(k == KI - 1),
                        )
                    nc.scalar.activation(
                        out=mixs[:, c * 512:(c + 1) * 512], in_=ps, func=AF.Copy
                    )
                for i in range(TT):
                    pt = pp_t.tile([128, 128], F32, tag="tpf", name="tpf")
                    nc.tensor.transpose(pt, mixs[:, i * 128:(i + 1) * 128], ident_f)
                    nc.vector.tensor_add(
                        out=a_tm[i][:, m * 128:(m + 1) * 128],
                        in0=u_tm[i][:, m * 128:(m + 1) * 128], in1=pt,
                    )

        # ---------------- LN2 (token-major) -> transpose -> MLP
        alnT = [persist.tile([128, L], BF16, tag=f"alnT{j}", name=f"alnT{j}") for j in range(KD)]
        with tc.tile_pool(name="ln2", bufs=5) as pool:
            for i in range(TT):
                st = pool.tile([128, 6], F32, tag="st", name="st")
                nc.vector.bn_stats(out=st, in_=a_tm[i])
                mv = pool.tile([128, 2], F32, tag="mv", name="mv")
                nc.vector.bn_aggr(out=mv, in_=st)
                nc.scalar.activation(
                    out=mv[:, 1:2], in_=mv[:, 1:2], func=AF.Sqrt, bias=eps_t
                )
                nc.vector.reciprocal(out=mv[:, 1:2], in_=mv[:, 1:2])
                aln = pool.tile([128, DIM], BF16, tag="aln", name="aln")
                nc.vector.tensor_scalar(
                    out=aln, in0=a_tm[i], scalar1=mv[:, 0:1], scalar2=mv[:, 1:2],
                    op0=OP.subtract, op1=OP.mult,
                )
                for j in range(KD):
                    pt = pp_t.tile([128, 128], BF16, tag="tpb", name="tpb")
                    nc.tensor.transpose(pt, aln[:, j * 128:(j + 1) * 128], ident_b)
                    nc.scalar.activation(
                        out=alnT[j][:, i * 128:(i + 1) * 128], in_=pt, func=AF.Copy
                    )
            for j in range(KD):
                nc.vector.tensor_scalar(
                    out=alnT[j], in0=alnT[j], scalar1=ln2w_s[j], scalar2=ln2b_s[j],
                    op0=OP.mult, op1=OP.add,
                )

        with tc.tile_pool(name="mlp", bufs=6) as pool:
            h1b = [pool.tile([96, L], BF16, tag=f"h1b{m2}", name=f"h1b{m2}") for m2 in range(2)]
            for m2 in range(2):
                for c in range(CH):
                    ps = pp_mm.tile([128, 512], F32, tag="mm", name="mm")
                    for k in range(KD):
                        nc.tensor.matmul(
                            ps[:96], fc1_s[k][:, m2 * 96:(m2 + 1) * 96],
                            alnT[k][:, c * 512:(c + 1) * 512],
                            start=(k == 0), stop=(k == KD - 1),
                        )
                    nc.scalar.activation(
                        out=h1b[m2][:, c * 512:(c + 1) * 512], in_=ps[:96],
                        func=AF.Gelu, bias=fc1b_s[m2],
                    )
            for m in range(KD):
                mls = pool.tile([128, L], F32, tag="mls", name="mls")
                for c in range(CH):
                    ps = pp_mm.tile([128, 512], F32, tag="mm", name="mm")
                    for k2 in range(2):
                        nc.tensor.matmul(
                            ps, fc2_s[k2][:, m * 128:(m + 1) * 128],
                            h1b[k2][:, c * 512:(c + 1) * 512],
                            start=(k2 == 0), stop=(k2 == 1),
                        )
                    nc.vector.tensor_scalar(
                        out=mls[:, c * 512:(c + 1) * 512], in0=ps,
                        scalar1=fc2b_s[m], scalar2=None, op0=OP.add,
                    )
                for i in range(TT):
                    pt = pp_t.tile([128, 128], F32, tag="tpf", name="tpf")
                    nc.tensor.transpose(pt, mls[:, i * 128:(i + 1) * 128], ident_f)
                    ot = pool.tile([128, 128], F32, tag="ot", name="ot")
                    nc.vector.tensor_add(
                        out=ot, in0=a_tm[i][:, m * 128:(m + 1) * 128], in1=pt
                    )
                    nc.sync.dma_start(
                        out=o_d[i * 128:(i + 1) * 128, m * 128:(m + 1) * 128],
                        in_=ot,
                    )

    _split_excess_waits(nc)
    return nc


def _prep_weights(inputs):
    f32 = np.float32
    w = {}
    w["w_in_t"] = np.ascontiguousarray(inputs["in_proj_w"].T).astype(BF)
    w["b_in"] = inputs["in_proj_b"].reshape(-1, 1).astype(f32)
    cw = inputs["conv_w"].reshape(9, DI).T        # (768, 9), tap k = dh*3+dw
    w["convw"] = np.ascontiguousarray(cw).astype(f32)
    w["convb"] = inputs["conv_b"].reshape(-1, 1).astype(f32)
    w["xp_t"] = np.ascontiguousarray(inputs["x_proj_w"].T).astype(BF)
    w["dtp_t"] = np.ascontiguousarray(inputs["dt_proj_w"].T).astype(BF)
    w["dtp_b"] = inputs["dt_proj_b"].reshape(-1, 1).astype(f32)
    w["d_skip"] = inputs["D"].reshape(-1, 1).astype(f32)
    w["onw"] = inputs["out_norm_w"].reshape(-1, 1).astype(f32)
    w["onb"] = inputs["out_norm_b"].reshape(-1, 1).astype(f32)
    w["op_t"] = np.ascontiguousarray(inputs["out_proj_w"].T).astype(BF)
    w["ln1w"] = inputs["ln1_w"].reshape(-1, 1).astype(f32)
    w["ln1b"] = inputs["ln1_b"].reshape(-1, 1).astype(f32)
    w["ln2w"] = inputs["ln2_w"].reshape(-1, 1).astype(f32)
    w["ln2b"] = inputs["ln2_b"].reshape(-1, 1).astype(f32)
    w["fc1_t"] = np.ascontiguousarray(inputs["fc1_w"].T).astype(BF)
    w["fc1_b"] = inputs["fc1_b"].reshape(-1, 1).astype(f32)
    w["fc2_t"] = np.ascontiguousarray(inputs["fc2_w"].T).astype(BF)
    w["fc2_b"] = inputs["fc2_b"].reshape(-1, 1).astype(f32)
    return w


def kernel(**inputs):
    _install_tilefix()
    inputs = {k: np.asarray(v) for k, v in inputs.items()}
    A = -np.exp(inputs["A_log"].astype(np.float64))   # (768, 16)
    assert np.abs(A - A[0:1, :]).max() < 1e-4, "A must be d-independent"
    a_coefs = tuple(float(v) for v in A[0])

    key = a_coefs
    if _CACHE.get("key") != key:
        _CACHE["nc"] = _build_program(a_coefs)
        _CACHE["key"] = key
    nc = _CACHE["nc"]

    w = _prep_weights(inputs)
    seqs = []
    for t in ("x", "y"):
        for b in range(4):
            seqs.append(
                np.ascontiguousarray(
                    inputs[t][b].reshape(L, DIM).astype(np.float32)
                )
            )
    in_maps = [dict(w, u=seqs[i]) for i in range(8)]
    res = run_bass_kernel_spmd(nc, in_maps, core_ids=list(range(8)))
    outs = [res.results[i]["o"].reshape(32, 32, DIM) for i in range(8)]
    a = np.stack(outs[:4]).astype(np.float32)
    b = np.stack(outs[4:]).astype(np.float32)
    return (a, b)

